# revision 1
# baseline (speedup 1.0000x reference)
"""GAT multi-head attention kernel for Trainium2 (8 NeuronCores, data-parallel over batch).

Problem (hardcoded): h [8,1024,128] f32, W [8,128,16] f32, Wa [8,32] f32.
  g   = einsum('bni,hid->hbnd', h, W)
  e   = leakyrelu(g@a_src [i] + g@a_dst [j], 0.2)      # [H,B,N,N]
  P   = softmax_j(e)
  out = relu(einsum('hbij,hbjd->bihd', P, g)).reshape(B,N,H*hd)

Sharding: graph b -> core b. Each core computes all 8 heads for its graph.

Algebra: with s=g@a_src (per-i), t=g@a_dst (per-j),
  exp(leakyrelu(s_i+t_j)) = max(e^{s_i+t_j}, e^{.2(s_i+t_j)})
                          = e^{.2 s_i} * e^{t_j} * max(e^{.8 s_i}, e^{-.8 t_j})
The e^{.2 s_i} factor is constant along the softmax axis j, so it cancels in
the softmax ratio and is simply dropped. The e^{t_j} factor rides along the
contraction dim of the attention matmul and is folded into the stationary
[g|1] (whose ones column also yields the softmax denominator). Each NxN
probability tile then costs ONE fused dual-op vector instruction:
  u'_ij = max(q_i, r_j) * F1_j,  q = e^{.8 s} (rows, DMA-broadcast across
  partitions), r = e^{-.8 t} and F1 = e^t (per-partition columns),
via tensor_scalar(op0=max, op1=mult) on the vector engine (bf16, 2x mode).
The attention matmuls pack 4 heads per PSUM accumulator at partition
offsets 32*hh (col-group tile_position), zero-padded stationaries keep all
128 partitions defined, and phase F does one [128,128] PE transpose per
(group, i-chunk) before a per-partition reciprocal-scale (relu folded in).
"""
import numpy as np
from contextlib import ExitStack

import concourse.bass as bass
import concourse.tile as tile
from concourse import bacc, mybir
from concourse import bass_utils

# ---- problem constants (from spec; kernel.py must be self-contained) ----
B, N, DI, H, HD = 8, 1024, 128, 8, 16
SLOPE = 0.2
NC128 = N // 128            # 8 chunks of 128
FP32 = mybir.dt.float32
BF16 = mybir.dt.bfloat16

AF = mybir.ActivationFunctionType
ALU = mybir.AluOpType

DT_MM = BF16          # dtype of probability tiles + attention-matmul operands

# ---- engine routing knobs ----
GPS_JCS = ()    # jc values whose u-tiles run on GPSIMD (never: ~10x slower)


def _use_act(hh, jc):
    # Offload a u-tile to the scalar engine (2-op relu identity) at most once
    # per 4-head round so the PE never waits on the slower ACT chain.
    return 1 <= jc <= 7 and hh == jc % 4


def build_nc(iters: int = 1, variant: str = "full"):
    nc = bacc.Bacc("TRN2", target_bir_lowering=False, debug=False, num_devices=8)

    hb_d = nc.dram_tensor("hb", [N, DI], FP32, kind="ExternalInput")
    wall_d = nc.dram_tensor("wall", [DI, H * HD], FP32, kind="ExternalInput")
    wabd_d = nc.dram_tensor("wabd", [DI, 2 * H], FP32, kind="ExternalInput")
    ident_d = nc.dram_tensor("ident", [128, 128], FP32, kind="ExternalInput")
    out_d = nc.dram_tensor("out", [N, H * HD], FP32, kind="ExternalOutput")

    with tile.TileContext(nc) as tc:
        with ExitStack() as ctx:
            if iters > 1:
                ctx.enter_context(tc.For_i(
                    0, iters, 1,
                    hint_engines=(mybir.EngineType.PE, mybir.EngineType.DVE,
                                  mybir.EngineType.Activation,
                                  mybir.EngineType.SP)))
            _body(ctx, tc, hb_d, wall_d, wabd_d, ident_d, out_d, variant)
    nc.compile()
    return nc


def _body(ctx, tc, hb_d, wall_d, wabd_d, ident_d, out_d, variant="full"):
    nc = tc.nc
    consts = ctx.enter_context(tc.tile_pool(name="consts", bufs=1))
    sb = ctx.enter_context(tc.tile_pool(name="sb", bufs=4))
    mtp = ctx.enter_context(tc.tile_pool(name="mtp", bufs=12))
    ps_small = ctx.enter_context(tc.tile_pool(name="ps_small", bufs=4, space="PSUM"))
    ps_oh = ctx.enter_context(tc.tile_pool(name="ps_oh", bufs=2, space="PSUM"))
    dram = ctx.enter_context(tc.tile_pool(name="dram", bufs=1, space="DRAM"))

    # ---- constants in ----
    ident = consts.tile([128, 128], FP32)
    nc.sync.dma_start(ident[:], ident_d.ap())
    wall = consts.tile([128, H * HD], FP32)
    nc.sync.dma_start(wall[:], wall_d.ap())
    wq = consts.tile([128, 2 * H], FP32)
    nc.sync.dma_start(wq[:], wabd_d.ap())

    # ---- phase A: load h (split DMAs), transpose to hT [128 i, 1024 n] ----
    hall = consts.tile([128, N], FP32)   # [p, c*128+i] = hb[c*128+p, i]
    for half in range(2):
        nc.sync.dma_start(
            hall[:, half * 512:(half + 1) * 512].rearrange(
                "p (c i) -> p c i", i=128),
            hb_d.ap()[half * 512:(half + 1) * 512, :].rearrange(
                "(c p) i -> p c i", p=128))
    hT = consts.tile([128, N], FP32)
    for icn in range(NC128):
        pt = ps_small.tile([128, 128], FP32, tag="ps", padded_shape=[128, 512])
        nc.tensor.transpose(pt[:], hall[:, icn * 128:(icn + 1) * 128], ident[:])
        nc.scalar.copy(hT[:, icn * 128:(icn + 1) * 128], pt[:])

    # ---- phase B: s rows straight from hT (wq = wall @ wabd, host-side),
    # then the broadcast chain, launched as early as possible ----
    srows = consts.tile([8, N], FP32)           # s_h(i) as rows
    for half in range(2):
        ps = ps_small.tile([8, 512], FP32, tag="ps", padded_shape=[128, 512])
        nc.tensor.matmul(ps[:], wq[:, 0:8], hT[:, half * 512:(half + 1) * 512],
                         start=True, stop=True)
        nc.scalar.copy(srows[:, half * 512:(half + 1) * 512], ps[:])

    # q rows = e^{0.8 s} -> bf16, bounce via DRAM, broadcast across partitions
    qrows = consts.tile([8, N], DT_MM)
    nc.scalar.activation(qrows[:], srows[:], AF.Exp, scale=0.8)
    qrows_d = dram.tile([H, N], DT_MM)
    nc.sync.dma_start(qrows_d[:], qrows[:])
    qb = consts.tile([128, H * N], DT_MM)
    for h in range(H):
        nc.sync.dma_start(qb[:, h * N:(h + 1) * N],
                          qrows_d[h:h + 1, :].partition_broadcast(128))

    # ---- phase C: st [128 n, jc*16 + (s_h | 8+t_h)] from hT, exp factors ----
    st = consts.tile([128, NC128 * 16], FP32)
    for jc in range(NC128):
        ps = ps_small.tile([128, 16], FP32, tag="ps", padded_shape=[128, 512])
        nc.tensor.matmul(ps[:], hT[:, jc * 128:(jc + 1) * 128], wq[:],
                         start=True, stop=True)
        nc.scalar.copy(st[:, jc * 16:(jc + 1) * 16], ps[:])

    t_view = st[:].rearrange("p (c q) -> p c q", q=16)[:, :, 8:16]
    # f1 = e^t (moving-side scaling), rcols = e^{-0.8 t} (u-tile scalar)
    f1 = consts.tile([128, NC128 * 8], FP32)
    nc.scalar.activation(f1[:].rearrange("p (c q) -> p c q", q=8), t_view, AF.Exp)
    rcols = consts.tile([128, NC128 * 8], FP32)
    nc.scalar.activation(rcols[:].rearrange("p (c q) -> p c q", q=8), t_view,
                         AF.Exp, scale=-0.8)
    f1r = consts.tile([128, NC128 * 8], FP32)   # e^{0.2 t} = f1 * rcols
    nc.scalar.activation(f1r[:].rearrange("p (c q) -> p c q", q=8), t_view,
                         AF.Exp, scale=SLOPE)
    rneg = consts.tile([128, NC128 * 8], FP32)  # -e^{-0.8 t}
    nc.vector.tensor_scalar(rneg[:], rcols[:], -1.0, None, ALU.mult)

    # ---- phase D: g_ext [128 j, jc*256 + h*32 + d]; col 16 = ones (den),
    # cols 17..31 zero padding so matmuls cover all 128 psum partitions ----
    g_ext = consts.tile([128, NC128 * 256], DT_MM)
    nc.vector.memset(g_ext[:], 0.0)
    ones_view = g_ext[:].rearrange("p (c q) -> p c q", q=32)[:, :, 16:17]
    nc.vector.memset(ones_view, 1.0)
    for jc in range(NC128):
        ps = ps_small.tile([128, 128], FP32, tag="ps", padded_shape=[128, 512])
        nc.tensor.matmul(ps[:], hT[:, jc * 128:(jc + 1) * 128], wall[:],
                         start=True, stop=True)
        dst = g_ext[:, jc * 256:(jc + 1) * 256].rearrange(
            "p (h q) -> p h q", q=32)[:, :, 0:16]
        src = ps[:].rearrange("p (h q) -> p h q", q=16)
        nc.scalar.copy(dst, src)

    # ---- phase E: u tiles + attention matmuls (4 heads packed per PSUM
    # accumulator at partition offsets 32*hh via col-group tile_position) ----
    o4s = [consts.tile([128, N], FP32, name=f"o4s{g}") for g in range(2)]
    outt_all = consts.tile([128, NC128 * H * HD], FP32)
    outt = [outt_all[:, i * H * HD:(i + 1) * H * HD] for i in range(NC128)]

    if variant == "ad":
        nc.vector.memset(outt_all[:], 0.0)
        nc.sync.dma_start(
            out_d.ap().rearrange("(c p) i -> p c i", p=128),
            outt_all[:].rearrange("p (c i) -> p c i", i=H * HD))
        return

    if variant == "ss":
        # scale stationary in place by e^t on ACT (incl. ones col)
        for jc in range(NC128):
            for h in range(H):
                sl = slice(jc * 256 + h * 32, jc * 256 + h * 32 + 17)
                nc.scalar.activation(g_ext[:, sl], g_ext[:, sl], AF.Copy,
                                     scale=f1[:, jc * 8 + h:jc * 8 + h + 1])

    for grp in range(2):
        o4 = ps_oh.tile([128, N], FP32, tag="oh")
        mts_shared = None
        if variant == "nodve":
            mts_shared = []
            for hh in range(4):
                h = grp * 4 + hh
                mt = mtp.tile([128, N], DT_MM, tag="mt", name=f"mts{hh}")
                nc.vector.tensor_scalar(mt[:], qb[:, h * N:(h + 1) * N],
                                        rcols[:, h:h + 1], f1[:, h:h + 1],
                                        ALU.max, ALU.mult)
                mts_shared.append(mt)
        for jc in range(NC128):
            if variant == "nodve":
                mts = mts_shared
            else:
                mts = []
                for hh in range(4):
                    h = grp * 4 + hh
                    mt = mtp.tile([128, N], DT_MM, tag="mt", name=f"mt{hh}")
                    c = jc * 8 + h
                    if _use_act(hh, jc) and variant == "full":
                        # mt = F1*(relu(q - r) + r) = F1*max(q, r) on ACT
                        vt = mtp.tile([128, N], DT_MM, tag="vt", name=f"vt{hh}",
                                      bufs=6)
                        nc.scalar.activation(vt[:], qb[:, h * N:(h + 1) * N],
                                             AF.Relu, bias=rneg[:, c:c + 1])
                        nc.scalar.activation(mt[:], vt[:], AF.Identity,
                                             scale=f1[:, c:c + 1],
                                             bias=f1r[:, c:c + 1])
                    elif variant == "ss":
                        nc.vector.tensor_scalar(mt[:], qb[:, h * N:(h + 1) * N],
                                                rcols[:, c:c + 1], None, ALU.max)
                    else:
                        nc.vector.tensor_scalar(mt[:], qb[:, h * N:(h + 1) * N],
                                                rcols[:, c:c + 1],
                                                f1[:, c:c + 1],
                                                ALU.max, ALU.mult)
                    mts.append(mt)
            # 4 heads' matmuls back-to-back at 4 col-groups -> they stream
            # concurrently on separate XBUSes (col-tiling concurrency)
            for ih in range(2):
                for hh in range(4):
                    h = grp * 4 + hh
                    lhsT = g_ext[:, jc * 256 + h * 32: jc * 256 + (h + 1) * 32]
                    nc.tensor.matmul(
                        o4[32 * hh:32 * hh + 32, ih * 512:(ih + 1) * 512],
                        lhsT, mts[hh][:, ih * 512:(ih + 1) * 512],
                        start=(jc == 0), stop=(jc == NC128 - 1),
                        tile_position=(0, 32 * hh), skip_group_check=True)
        for ih in range(2):
            nc.scalar.copy(o4s[grp][:, ih * 512:(ih + 1) * 512],
                           o4[:, ih * 512:(ih + 1) * 512])

    # ---- phase F: one [128,128] transpose per (grp, i-chunk), normalize ----
    for grp in range(2):
        for icn in range(NC128):
            t4 = ps_small.tile([128, 128], FP32, tag="ps",
                               padded_shape=[128, 512])
            nc.tensor.transpose(
                t4[:], o4s[grp][:, icn * 128:(icn + 1) * 128], ident[:])
            r4 = sb.tile([128, 4], FP32, tag="r4")
            den = t4[:].rearrange("p (a q) -> p a q", q=32)[:, :, 16:17]
            nc.vector.reciprocal(r4[:].rearrange("p (a q) -> p a q", q=1), den)
            for hh in range(4):
                h = grp * 4 + hh
                # relu(r*num) == relu(num)/den since den>0
                nc.scalar.activation(outt[icn][:, h * HD:(h + 1) * HD],
                                     t4[:, 32 * hh:32 * hh + 16], AF.Relu,
                                     scale=r4[:, hh:hh + 1])

    nc.sync.dma_start(
        out_d.ap().rearrange("(c p) i -> p c i", p=128),
        outt_all[:].rearrange("p (c i) -> p c i", i=H * HD))


# ---- host wrapper ----
_CACHE = {}


def _prep_weights(W, Wa):
    W = np.asarray(W, dtype=np.float32)
    Wa = np.asarray(Wa, dtype=np.float32)
    wall = np.ascontiguousarray(W.transpose(1, 0, 2).reshape(DI, H * HD))
    wabd = np.zeros((DI, 2 * H), dtype=np.float32)
    for hh in range(H):
        wabd[hh * HD:(hh + 1) * HD, hh] = Wa[hh, :HD]
        wabd[hh * HD:(hh + 1) * HD, H + hh] = Wa[hh, HD:]
    wq = np.ascontiguousarray(wall @ wabd)   # s/t projections direct from hT
    ident = np.eye(128, dtype=np.float32)
    return wall, wq, ident


def kernel(h, W, Wa):
    h = np.asarray(h, dtype=np.float32)
    if "nc" not in _CACHE:
        _CACHE["nc"] = build_nc(iters=1)
    nc = _CACHE["nc"]
    wall, wabd, ident = _prep_weights(W, Wa)
    in_maps = [
        {"hb": np.ascontiguousarray(h[c]), "wall": wall, "wabd": wabd,
         "ident": ident}
        for c in range(B)
    ]
    res = bass_utils.run_bass_kernel_spmd(nc, in_maps, core_ids=list(range(B)))
    out = np.stack([res.results[c]["out"] for c in range(B)], axis=0)
    return out.astype(np.float32)



# revision 13
# speedup vs baseline: 1.4007x; 1.4007x over previous
"""GAT multi-head attention kernel for Trainium2 (8 NeuronCores, data-parallel over batch).

Problem (hardcoded): h [8,1024,128] f32, W [8,128,16] f32, Wa [8,32] f32.
  g   = einsum('bni,hid->hbnd', h, W)
  e   = leakyrelu(g@a_src [i] + g@a_dst [j], 0.2)      # [H,B,N,N]
  P   = softmax_j(e)
  out = relu(einsum('hbij,hbjd->bihd', P, g)).reshape(B,N,H*hd)

Sharding: graph b -> core b. Each core computes all 8 heads for its graph.

Algebra: with s=g@a_src (per-i), t=g@a_dst (per-j),
  exp(leakyrelu(s_i+t_j)) = e^{0.2 s_i} * max(e^{t_j} e^{0.8 s_i}, e^{0.2 t_j})
The e^{0.2 s_i} factor is constant along the softmax axis j and cancels in the
softmax ratio, so it is dropped. Each unnormalized probability tile is ONE
dual-op DVE instruction (bf16, 4x mode):
  u_ij = max(F1_j * q_i, f1r_j),  q = e^{0.8 s} (rows, DMA-broadcast across
  partitions), F1 = e^t and f1r = e^{0.2 t} (per-partition scalars),
via tensor_scalar(op0=mult, op1=max). The attention matmuls pack 4 heads per
PSUM accumulator at partition offsets 32*hh (col-group tile_position) with a
[g|1] stationary whose ones column yields the softmax denominator. Phase F
batches 4 transposes per PSUM bank and fuses relu+normalize into one
scalar_tensor_tensor per 512-row half, reading PSUM directly; grp0's phase F
interleaves with grp1's phase E so only grp1's epilogue is kernel tail.
The s/t projections use wq = wall @ wabd (host-side).
All constant weights arrive in one DMA (wconst = wall|wq|ident); q rows are
broadcast across partitions with per-head DMAs via a DRAM bounce.
"""
import numpy as np
from contextlib import ExitStack

import concourse.bass as bass
import concourse.tile as tile
from concourse import bacc, mybir
from concourse import bass_utils

# ---- problem constants (from spec; kernel.py must be self-contained) ----
B, N, DI, H, HD = 8, 1024, 128, 8, 16
SLOPE = 0.2
NC128 = N // 128            # 8 chunks of 128
FP32 = mybir.dt.float32
FP32R = mybir.dt.float32r
BF16 = mybir.dt.bfloat16

AF = mybir.ActivationFunctionType
ALU = mybir.AluOpType

DT_MM = BF16          # dtype of probability tiles + attention-matmul operands
WC = H * HD + 2 * H + 128      # wconst cols: wall | wq | ident


def build_nc(iters: int = 1, variant: str = "full"):
    nc = bacc.Bacc("TRN2", target_bir_lowering=False, debug=False, num_devices=8)

    hb_d = nc.dram_tensor("hb", [N, DI], FP32, kind="ExternalInput")
    wconst_d = nc.dram_tensor("wconst", [DI, WC], FP32, kind="ExternalInput")
    out_d = nc.dram_tensor("out", [N, H * HD], FP32, kind="ExternalOutput")

    with tile.TileContext(nc) as tc:
        with ExitStack() as ctx:
            if iters > 1:
                ctx.enter_context(tc.For_i(
                    0, iters, 1,
                    hint_engines=(mybir.EngineType.PE, mybir.EngineType.DVE,
                                  mybir.EngineType.Activation,
                                  mybir.EngineType.Pool,
                                  mybir.EngineType.SP)))
            _body(ctx, tc, hb_d, wconst_d, out_d, variant)
    nc.compile()
    return nc


def _body(ctx, tc, hb_d, wconst_d, out_d, variant="full"):
    nc = tc.nc
    consts = ctx.enter_context(tc.tile_pool(name="consts", bufs=1))
    mtp = ctx.enter_context(tc.tile_pool(name="mtp", bufs=16))
    ps_small = ctx.enter_context(tc.tile_pool(name="ps_small", bufs=4, space="PSUM"))
    ps_oh = ctx.enter_context(tc.tile_pool(name="ps_oh", bufs=2, space="PSUM"))
    dram = ctx.enter_context(tc.tile_pool(name="dram", bufs=1, space="DRAM"))

    # ---- inputs: one wconst DMA + h in two halves ----
    wc = consts.tile([128, WC], FP32)
    nc.sync.dma_start(wc[:], wconst_d.ap())
    wall = wc[:, 0:H * HD]
    wq = wc[:, H * HD:H * HD + 2 * H]
    ident = wc[:, H * HD + 2 * H:]

    hall = consts.tile([128, N], FP32)   # [p, c*128+i] = hb[c*128+p, i]
    for lo, hi in ((0, 256), (256, 512), (512, 1024)):
        nc.sync.dma_start(
            hall[:, lo:hi].rearrange("p (c i) -> p c i", i=128),
            hb_d.ap()[lo:hi, :].rearrange("(c p) i -> p c i", p=128))

    hT = consts.tile([128, N], FP32)
    srows = consts.tile([8, N], FP32)           # s_h(i) as rows
    qrows = consts.tile([8, N], DT_MM)          # e^{0.8 s}
    qrows_d = dram.tile([H, N], DT_MM)
    qb = consts.tile([128, H * N], DT_MM)

    # A-transposes chunks 0-3, then srows half 0, then chunks 4-7, half 1.
    # Per half: srows matmul, copy, exp, store to DRAM; then per-head
    # partition-broadcast DMAs (head order, so grp0 unblocks first).
    for half in range(2):
        for icn in range(half * 4, half * 4 + 4):
            pt = ps_small.tile([128, 128], FP32, tag="ps", padded_shape=[128, 512])
            nc.tensor.transpose(pt[:], hall[:, icn * 128:(icn + 1) * 128], ident)
            nc.scalar.copy(hT[:, icn * 128:(icn + 1) * 128], pt[:])
        ps = ps_small.tile([8, 512], FP32, tag="ps", padded_shape=[128, 512])
        nc.tensor.matmul(ps[:], wq[:, 0:8],
                         hT[:, half * 512:(half + 1) * 512],
                         start=True, stop=True)
        nc.scalar.copy(srows[:, half * 512:(half + 1) * 512], ps[:])
        nc.scalar.activation(qrows[:, half * 512:(half + 1) * 512],
                             srows[:, half * 512:(half + 1) * 512], AF.Exp,
                             scale=0.8)
        nc.sync.dma_start(qrows_d[:, half * 512:(half + 1) * 512],
                          qrows[:, half * 512:(half + 1) * 512])
    for h in range(H):
        nc.sync.dma_start(qb[:, h * N:(h + 1) * N],
                          qrows_d[h:h + 1, :].partition_broadcast(128))

    # ---- phase C: st [128 n, jc*16 + (s_h | 8+t_h)] from hT, exp factors ----
    st = consts.tile([128, NC128 * 16], FP32)
    for jc in range(NC128):
        ps = ps_small.tile([128, 16], FP32, tag="ps", padded_shape=[128, 512])
        nc.tensor.matmul(ps[:], hT[:, jc * 128:(jc + 1) * 128],
                         wq, start=True, stop=True)
        nc.scalar.copy(st[:, jc * 16:(jc + 1) * 16], ps[:])

    t_view = st[:].rearrange("p (c q) -> p c q", q=16)[:, :, 8:16]
    # f1 = e^t (mult factor), f1r = e^{0.2 t} (max bound): u = max(f1*q, f1r)
    f1 = consts.tile([128, NC128 * 8], FP32)
    nc.scalar.activation(f1[:].rearrange("p (c q) -> p c q", q=8), t_view, AF.Exp)
    f1r = consts.tile([128, NC128 * 8], FP32)
    nc.scalar.activation(f1r[:].rearrange("p (c q) -> p c q", q=8), t_view,
                         AF.Exp, scale=SLOPE)

    # ---- phase D: g_ext [128 j, jc*256 + h*32 + d]; col 16 = ones (den),
    # cols 17..31 zero padding so matmuls cover all 128 psum partitions ----
    g_ext = consts.tile([128, NC128 * 256], DT_MM)
    nc.gpsimd.memset(g_ext[:], 0.0)
    ones_view = g_ext[:].rearrange("p (c q) -> p c q", q=32)[:, :, 16:17]
    nc.gpsimd.memset(ones_view, 1.0)
    for jc in range(NC128):
        ps = ps_small.tile([128, 128], FP32, tag="ps", padded_shape=[128, 512])
        nc.tensor.matmul(ps[:], hT[:, jc * 128:(jc + 1) * 128], wall,
                         start=True, stop=True)
        dst = g_ext[:, jc * 256:(jc + 1) * 256].rearrange(
            "p (h q) -> p h q", q=32)[:, :, 0:16]
        src = ps[:].rearrange("p (h q) -> p h q", q=16)
        nc.scalar.copy(dst, src)

    # ---- phase E: u tiles + attention matmuls (4 heads packed per PSUM
    # accumulator at partition offsets 32*hh via col-group tile_position);
    # phase F (transpose + fused relu/normalize) interleaved per group ----
    o4s = [consts.tile([128, N], FP32, name=f"o4s{g}") for g in range(2)]
    outt_all = consts.tile([128, NC128 * H * HD], FP32)
    o4 = [None, None]
    tps = {}     # (grp, half) -> [128,512] PSUM transpose collection tile
    r4s = {}     # (grp, half) -> [128,16] reciprocal denominators

    def emit_E(grp, jcs):
        for jc in jcs:
            mts = []
            for hh in range(4):
                h = grp * 4 + hh
                c = jc * 8 + h
                mt = mtp.tile([128, N], DT_MM, tag="mt", name=f"mt{hh}")
                nc.vector.tensor_scalar(mt[:], qb[:, h * N:(h + 1) * N],
                                        f1[:, c:c + 1], f1r[:, c:c + 1],
                                        ALU.mult, ALU.max)
                mts.append(mt)
            # 4 heads' matmuls back-to-back at 4 col-groups (col-tiling
            # concurrency on HW)
            for ih in range(2):
                for hh in range(4):
                    h = grp * 4 + hh
                    lhsT = g_ext[:, jc * 256 + h * 32: jc * 256 + (h + 1) * 32]
                    nc.tensor.matmul(
                        o4[grp][32 * hh:32 * hh + 32, ih * 512:(ih + 1) * 512],
                        lhsT, mts[hh][:, ih * 512:(ih + 1) * 512],
                        start=(jc == 0), stop=(jc == NC128 - 1),
                        tile_position=(0, 32 * hh), skip_group_check=True)

    def emit_F_copy(grp, ih):
        # o4 PSUM -> o4s SBUF; ih half == transpose half (i columns)
        nc.scalar.copy(o4s[grp][:, ih * 512:(ih + 1) * 512],
                       o4[grp][:, ih * 512:(ih + 1) * 512])

    def emit_F_trans(grp, half):
        t4 = ps_small.tile([128, 512], FP32, tag="ps", padded_shape=[128, 512],
                           name=f"t4_{grp}_{half}")
        tps[(grp, half)] = t4
        for c in range(4):
            icn = half * 4 + c
            nc.tensor.transpose(t4[:, c * 128:(c + 1) * 128],
                                o4s[grp][:, icn * 128:(icn + 1) * 128],
                                ident)

    def emit_F_norm(grp, half):
        # t4 free layout: (c 4, hh 4, 32) with cols 0..15 = numerator,
        # col 16 = denominator.  out = relu(num) * (1/den).
        t4 = tps[(grp, half)]
        t4v = t4[:].rearrange("p (c h q) -> p c h q", h=4, q=32)
        r4 = consts.tile([128, 16], FP32, name=f"r4_{grp}_{half}")
        r4s[(grp, half)] = r4
        nc.vector.reciprocal(r4[:].rearrange("p (c h) -> p c h", h=4),
                             t4v[:, :, :, 16:17])
        dst = outt_all[:].rearrange("p (c h q) -> p c h q", h=8, q=16)[
            :, half * 4:(half + 1) * 4, grp * 4:(grp + 1) * 4, :]
        rrep = r4[:].rearrange("p (c h) -> p c h", h=4).broadcast_to(
            [128, 4, 4, 16])
        nc.vector.scalar_tensor_tensor(dst, t4v[:, :, :, 0:16], 0.0, rrep,
                                       ALU.max, ALU.mult)

    o4[0] = ps_oh.tile([128, N], FP32, tag="oh", name="o4g0")
    emit_E(0, range(2))
    o4[1] = ps_oh.tile([128, N], FP32, tag="oh", name="o4g1")
    emit_E(0, range(2, NC128))
    emit_F_copy(0, 0)
    emit_F_copy(0, 1)
    emit_E(1, range(2))
    emit_F_trans(0, 0)
    emit_F_trans(0, 1)
    emit_E(1, [2])
    emit_F_norm(0, 0)
    emit_E(1, [3])
    emit_F_norm(0, 1)
    emit_E(1, range(4, NC128))
    emit_F_copy(1, 0)
    emit_F_trans(1, 0)
    emit_F_copy(1, 1)
    emit_F_norm(1, 0)
    # output rows half*512..: needs norm(0,half) and norm(1,half)
    nc.sync.dma_start(
        out_d.ap()[0:512, :].rearrange("(c p) i -> p c i", p=128),
        outt_all[:, 0:512].rearrange("p (c i) -> p c i", i=H * HD))
    emit_F_trans(1, 1)
    emit_F_norm(1, 1)
    nc.sync.dma_start(
        out_d.ap()[512:1024, :].rearrange("(c p) i -> p c i", p=128),
        outt_all[:, 512:1024].rearrange("p (c i) -> p c i", i=H * HD))


# ---- host wrapper ----
_CACHE = {}


def _prep_weights(W, Wa):
    W = np.asarray(W, dtype=np.float32)
    Wa = np.asarray(Wa, dtype=np.float32)
    wall = np.ascontiguousarray(W.transpose(1, 0, 2).reshape(DI, H * HD))
    wabd = np.zeros((DI, 2 * H), dtype=np.float32)
    for hh in range(H):
        wabd[hh * HD:(hh + 1) * HD, hh] = Wa[hh, :HD]
        wabd[hh * HD:(hh + 1) * HD, H + hh] = Wa[hh, HD:]
    wq = np.ascontiguousarray(wall @ wabd)   # s/t projections direct from hT
    ident = np.eye(128, dtype=np.float32)
    wconst = np.ascontiguousarray(
        np.concatenate([wall, wq, ident], axis=1))   # [128, 272]
    return wconst


def kernel(h, W, Wa):
    h = np.asarray(h, dtype=np.float32)
    if "nc" not in _CACHE:
        _CACHE["nc"] = build_nc(iters=1)
    nc = _CACHE["nc"]
    wconst = _prep_weights(W, Wa)
    in_maps = [
        {"hb": np.ascontiguousarray(h[c]), "wconst": wconst}
        for c in range(B)
    ]
    res = bass_utils.run_bass_kernel_spmd(nc, in_maps, core_ids=list(range(B)))
    out = np.stack([res.results[c]["out"] for c in range(B)], axis=0)
    return out.astype(np.float32)


# revision 15
# speedup vs baseline: 1.4742x; 1.0525x over previous
"""GAT multi-head attention kernel for Trainium2 (8 NeuronCores, data-parallel over batch).

Problem (hardcoded): h [8,1024,128] f32, W [8,128,16] f32, Wa [8,32] f32.
  g   = einsum('bni,hid->hbnd', h, W)
  e   = leakyrelu(g@a_src [i] + g@a_dst [j], 0.2)      # [H,B,N,N]
  P   = softmax_j(e)
  out = relu(einsum('hbij,hbjd->bihd', P, g)).reshape(B,N,H*hd)

Sharding: graph b -> core b. Each core computes all 8 heads for its graph.

Algebra: with s=g@a_src (per-i), t=g@a_dst (per-j),
  exp(leakyrelu(s_i+t_j)) = e^{0.2 s_i} * max(e^{t_j} e^{0.8 s_i}, e^{0.2 t_j})
The e^{0.2 s_i} factor is constant along the softmax axis j and cancels in the
softmax ratio, so it is dropped. Each unnormalized probability tile is ONE
dual-op DVE instruction (bf16, 4x mode):
  u_ij = max(F1_j * q_i, f1r_j),  q = e^{0.8 s} (rows, DMA-broadcast across
  partitions), F1 = e^t and f1r = e^{0.2 t} (per-partition scalars),
via tensor_scalar(op0=mult, op1=max). The attention matmuls pack 4 heads per
PSUM accumulator at partition offsets 32*hh (col-group tile_position) with a
[g|1] stationary whose ones column yields the softmax denominator. Phase F
batches 4 transposes per PSUM bank and fuses relu+normalize into one
scalar_tensor_tensor per 512-row half, reading PSUM directly; grp0's phase F
interleaves with grp1's phase E so only grp1's epilogue is kernel tail.
The s/t projections use wq = wall @ wabd (host-side).
All constant weights arrive in one DMA (wconst = wall|wq|ident); q rows are
broadcast across partitions with per-head DMAs via a DRAM bounce.
"""
import numpy as np
from contextlib import ExitStack

import concourse.bass as bass
import concourse.tile as tile
from concourse import bacc, mybir
from concourse import bass_utils

# ---- problem constants (from spec; kernel.py must be self-contained) ----
B, N, DI, H, HD = 8, 1024, 128, 8, 16
SLOPE = 0.2
NC128 = N // 128            # 8 chunks of 128
FP32 = mybir.dt.float32
FP32R = mybir.dt.float32r
BF16 = mybir.dt.bfloat16

AF = mybir.ActivationFunctionType
ALU = mybir.AluOpType

DT_MM = BF16          # dtype of probability tiles + attention-matmul operands
WC = H * HD + 2 * H + 128      # wconst cols: wall | wq | ident


def build_nc(iters: int = 1, variant: str = "full"):
    nc = bacc.Bacc("TRN2", target_bir_lowering=False, debug=False, num_devices=8)

    hb_d = nc.dram_tensor("hb", [N, DI], FP32, kind="ExternalInput")
    wconst_d = nc.dram_tensor("wconst", [DI, WC], FP32, kind="ExternalInput")
    out_d = nc.dram_tensor("out", [N, H * HD], FP32, kind="ExternalOutput")

    with tile.TileContext(nc) as tc:
        with ExitStack() as ctx:
            if iters > 1:
                ctx.enter_context(tc.For_i(
                    0, iters, 1,
                    hint_engines=(mybir.EngineType.PE, mybir.EngineType.DVE,
                                  mybir.EngineType.Activation,
                                  mybir.EngineType.Pool,
                                  mybir.EngineType.SP)))
            _body(ctx, tc, hb_d, wconst_d, out_d, variant)
    nc.compile()
    return nc


def _body(ctx, tc, hb_d, wconst_d, out_d, variant="full"):
    nc = tc.nc
    consts = ctx.enter_context(tc.tile_pool(name="consts", bufs=1))
    mtp = ctx.enter_context(tc.tile_pool(name="mtp", bufs=16))
    ps_small = ctx.enter_context(tc.tile_pool(name="ps_small", bufs=4, space="PSUM"))
    ps_oh = ctx.enter_context(tc.tile_pool(name="ps_oh", bufs=2, space="PSUM"))
    dram = ctx.enter_context(tc.tile_pool(name="dram", bufs=1, space="DRAM"))

    # ---- inputs: one wconst DMA + h in two halves ----
    wc = consts.tile([128, WC], FP32)
    nc.sync.dma_start(wc[:], wconst_d.ap())
    wall = wc[:, 0:H * HD]
    wq = wc[:, H * HD:H * HD + 2 * H]
    ident = wc[:, H * HD + 2 * H:]

    hall = consts.tile([128, N], FP32)   # [p, c*128+i] = hb[c*128+p, i]
    for lo, hi in ((0, 256), (256, 512), (512, 1024)):
        nc.sync.dma_start(
            hall[:, lo:hi].rearrange("p (c i) -> p c i", i=128),
            hb_d.ap()[lo:hi, :].rearrange("(c p) i -> p c i", p=128))

    hT = consts.tile([128, N], FP32)
    srows = consts.tile([8, N], FP32)           # s_h(i) as rows
    qrows = consts.tile([8, N], DT_MM)          # e^{0.8 s}
    qrows_d = dram.tile([H, N], DT_MM)
    qb = consts.tile([128, H * N], DT_MM)

    # A-transposes chunks 0-3, then srows half 0, then chunks 4-7, half 1.
    # Per half: srows matmul, copy, exp, store to DRAM; then per-head
    # partition-broadcast DMAs (head order, so grp0 unblocks first).
    for half in range(2):
        for icn in range(half * 4, half * 4 + 4):
            pt = ps_small.tile([128, 128], FP32, tag="ps", padded_shape=[128, 512])
            nc.tensor.transpose(pt[:], hall[:, icn * 128:(icn + 1) * 128], ident)
            nc.scalar.copy(hT[:, icn * 128:(icn + 1) * 128], pt[:])
        ps = ps_small.tile([8, 512], FP32, tag="ps", padded_shape=[128, 512])
        nc.tensor.matmul(ps[:], wq[:, 0:8],
                         hT[:, half * 512:(half + 1) * 512],
                         start=True, stop=True)
        nc.scalar.copy(srows[:, half * 512:(half + 1) * 512], ps[:])
        nc.scalar.activation(qrows[:, half * 512:(half + 1) * 512],
                             srows[:, half * 512:(half + 1) * 512], AF.Exp,
                             scale=0.8)
        nc.sync.dma_start(qrows_d[:, half * 512:(half + 1) * 512],
                          qrows[:, half * 512:(half + 1) * 512])
    for h in range(H):
        nc.sync.dma_start(qb[:, h * N:(h + 1) * N],
                          qrows_d[h:h + 1, :].partition_broadcast(128))

    # ---- phase C: st [128 n, jc*16 + (s_h | 8+t_h)] from hT, exp factors ----
    st = consts.tile([128, NC128 * 16], FP32)
    for jc in range(NC128):
        ps = ps_small.tile([128, 16], FP32, tag="ps", padded_shape=[128, 512])
        nc.tensor.matmul(ps[:], hT[:, jc * 128:(jc + 1) * 128],
                         wq, start=True, stop=True)
        nc.scalar.copy(st[:, jc * 16:(jc + 1) * 16], ps[:])

    t_view = st[:].rearrange("p (c q) -> p c q", q=16)[:, :, 8:16]
    # f1 = e^t (mult factor), f1r = e^{0.2 t} (max bound): u = max(f1*q, f1r)
    f1 = consts.tile([128, NC128 * 8], FP32)
    nc.scalar.activation(f1[:].rearrange("p (c q) -> p c q", q=8), t_view, AF.Exp)
    f1r = consts.tile([128, NC128 * 8], FP32)
    nc.scalar.activation(f1r[:].rearrange("p (c q) -> p c q", q=8), t_view,
                         AF.Exp, scale=SLOPE)

    # ---- phase D: g_ext [128 j, jc*256 + h*32 + d]; col 16 = ones (den),
    # cols 17..31 zero padding so matmuls cover all 128 psum partitions ----
    g_ext = consts.tile([128, NC128 * 256], DT_MM)
    nc.gpsimd.memset(g_ext[:], 0.0)
    ones_view = g_ext[:].rearrange("p (c q) -> p c q", q=32)[:, :, 16:17]
    nc.gpsimd.memset(ones_view, 1.0)
    for jc in range(NC128):
        ps = ps_small.tile([128, 128], FP32, tag="ps", padded_shape=[128, 512])
        nc.tensor.matmul(ps[:], hT[:, jc * 128:(jc + 1) * 128], wall,
                         start=True, stop=True)
        dst = g_ext[:, jc * 256:(jc + 1) * 256].rearrange(
            "p (h q) -> p h q", q=32)[:, :, 0:16]
        src = ps[:].rearrange("p (h q) -> p h q", q=16)
        nc.scalar.copy(dst, src)

    # ---- phase E: u tiles + attention matmuls (4 heads packed per PSUM
    # accumulator at partition offsets 32*hh via col-group tile_position);
    # phase F (transpose + fused relu/normalize) interleaved per group ----
    o4s = [consts.tile([128, N], FP32, name=f"o4s{g}") for g in range(2)]
    outt_all = consts.tile([128, NC128 * H * HD], FP32)
    o4 = [None, None]
    tps = {}     # (grp, half) -> [128,512] PSUM transpose collection tile
    r4s = {}     # (grp, half) -> [128,16] reciprocal denominators

    def emit_E(grp, jcs, split=False):
        for jc in jcs:
            mts = [mtp.tile([128, N], DT_MM, tag="mt", name=f"mt{hh}")
                   for hh in range(4)]
            for lo, hi in ((0, N),) if not split else ((0, 512), (512, N)):
                for hh in range(4):
                    h = grp * 4 + hh
                    c = jc * 8 + h
                    nc.vector.tensor_scalar(
                        mts[hh][:, lo:hi], qb[:, h * N + lo: h * N + hi],
                        f1[:, c:c + 1], f1r[:, c:c + 1], ALU.mult, ALU.max)
            # 4 heads' matmuls back-to-back at 4 col-groups (col-tiling
            # concurrency on HW)
            for ih in range(2):
                for hh in range(4):
                    h = grp * 4 + hh
                    lhsT = g_ext[:, jc * 256 + h * 32: jc * 256 + (h + 1) * 32]
                    nc.tensor.matmul(
                        o4[grp][32 * hh:32 * hh + 32, ih * 512:(ih + 1) * 512],
                        lhsT, mts[hh][:, ih * 512:(ih + 1) * 512],
                        start=(jc == 0), stop=(jc == NC128 - 1),
                        tile_position=(0, 32 * hh), skip_group_check=True)

    def emit_F_copy(grp, ih):
        # o4 PSUM -> o4s SBUF; ih half == transpose half (i columns)
        nc.scalar.copy(o4s[grp][:, ih * 512:(ih + 1) * 512],
                       o4[grp][:, ih * 512:(ih + 1) * 512])

    def emit_F_trans(grp, half):
        t4 = ps_small.tile([128, 512], FP32, tag="ps", padded_shape=[128, 512],
                           name=f"t4_{grp}_{half}")
        tps[(grp, half)] = t4
        for c in range(4):
            icn = half * 4 + c
            nc.tensor.transpose(t4[:, c * 128:(c + 1) * 128],
                                o4s[grp][:, icn * 128:(icn + 1) * 128],
                                ident)

    def emit_F_norm(grp, half):
        # t4 free layout: (c 4, hh 4, 32) with cols 0..15 = numerator,
        # col 16 = denominator.  out = relu(num) * (1/den).
        t4 = tps[(grp, half)]
        t4v = t4[:].rearrange("p (c h q) -> p c h q", h=4, q=32)
        r4 = consts.tile([128, 16], FP32, name=f"r4_{grp}_{half}")
        r4s[(grp, half)] = r4
        nc.vector.reciprocal(r4[:].rearrange("p (c h) -> p c h", h=4),
                             t4v[:, :, :, 16:17])
        dst = outt_all[:].rearrange("p (c h q) -> p c h q", h=8, q=16)[
            :, half * 4:(half + 1) * 4, grp * 4:(grp + 1) * 4, :]
        rrep = r4[:].rearrange("p (c h) -> p c h", h=4).broadcast_to(
            [128, 4, 4, 16])
        nc.vector.scalar_tensor_tensor(dst, t4v[:, :, :, 0:16], 0.0, rrep,
                                       ALU.max, ALU.mult)

    o4[0] = ps_oh.tile([128, N], FP32, tag="oh", name="o4g0")
    emit_E(0, range(2))
    o4[1] = ps_oh.tile([128, N], FP32, tag="oh", name="o4g1")
    emit_E(0, range(2, NC128))
    emit_F_copy(0, 0)
    emit_F_copy(0, 1)
    emit_E(1, range(2))
    emit_F_trans(0, 0)
    emit_F_trans(0, 1)
    emit_E(1, [2])
    emit_F_norm(0, 0)
    emit_E(1, [3])
    emit_F_norm(0, 1)
    emit_E(1, range(4, NC128))
    emit_F_copy(1, 0)
    emit_F_trans(1, 0)
    emit_F_copy(1, 1)
    emit_F_norm(1, 0)
    # output rows half*512..: needs norm(0,half) and norm(1,half)
    nc.sync.dma_start(
        out_d.ap()[0:512, :].rearrange("(c p) i -> p c i", p=128),
        outt_all[:, 0:512].rearrange("p (c i) -> p c i", i=H * HD))
    emit_F_trans(1, 1)
    emit_F_norm(1, 1)
    nc.sync.dma_start(
        out_d.ap()[512:1024, :].rearrange("(c p) i -> p c i", p=128),
        outt_all[:, 512:1024].rearrange("p (c i) -> p c i", i=H * HD))


# ---- host wrapper ----
_CACHE = {}


def _prep_weights(W, Wa):
    W = np.asarray(W, dtype=np.float32)
    Wa = np.asarray(Wa, dtype=np.float32)
    wall = np.ascontiguousarray(W.transpose(1, 0, 2).reshape(DI, H * HD))
    wabd = np.zeros((DI, 2 * H), dtype=np.float32)
    for hh in range(H):
        wabd[hh * HD:(hh + 1) * HD, hh] = Wa[hh, :HD]
        wabd[hh * HD:(hh + 1) * HD, H + hh] = Wa[hh, HD:]
    wq = np.ascontiguousarray(wall @ wabd)   # s/t projections direct from hT
    ident = np.eye(128, dtype=np.float32)
    wconst = np.ascontiguousarray(
        np.concatenate([wall, wq, ident], axis=1))   # [128, 272]
    return wconst


def kernel(h, W, Wa):
    h = np.asarray(h, dtype=np.float32)
    if "nc" not in _CACHE:
        _CACHE["nc"] = build_nc(iters=1)
    nc = _CACHE["nc"]
    wconst = _prep_weights(W, Wa)
    in_maps = [
        {"hb": np.ascontiguousarray(h[c]), "wconst": wconst}
        for c in range(B)
    ]
    res = bass_utils.run_bass_kernel_spmd(nc, in_maps, core_ids=list(range(B)))
    out = np.stack([res.results[c]["out"] for c in range(B)], axis=0)
    return out.astype(np.float32)
